# revision 13
# baseline (speedup 1.0000x reference)
"""Binarized MLP forward (BinaryConnect, training-mode BatchNorm) on 8 TRN2 cores.

Strategy: data-parallel over the batch (16384 -> 8 x 2048), weights replicated.
All activations kept TRANSPOSED on device ([features, batch]) so that
 - matmuls use binarized weights as the stationary operand,
 - BatchNorm stats are free-axis reductions (bn_stats on VectorE),
 - BN apply + ReLU is a per-partition scale/bias op (ScalarE ACT / VectorE TS).

v4 schedule notes (from v1-v3 traces):
 - Weights are binarized on the HOST and shipped as +-1 f32; w1 rides the
   scalar-engine DMA queue concurrently with x on the sync queue, w2/w3 queue
   on sync BEHIND x so they cannot starve the layer-1 stream.
 - L1 runs in four batch passes (j-outer, k-inner inside a pass) so each pass
   only needs 1.6MB of x; its stats all-reduces are CC-init/skew-bound anyway.
 - Two stat groups per layer; group-a's AR triggers ~75% into the layer,
   group-b's AR rides the tail, hidden by the next layer's leading k-phase
   (first two output tiles, PSUM-bank-limited).  L3 splits (5,3) so its
   group-a AR lands earlier ahead of the small L4.
 - BN applies are produced on ScalarE and VectorE in parallel; the next
   layer's phase loop consumes k-interleaved across two output tiles so one
   applied chunk feeds two matmuls back-to-back.
 - AR-gated work is emitted after the last feature-tile close of its layer so
   engine FIFOs cannot head-of-line-block late stats behind AR waits.
Matmuls run in float32r (full PE rate at N=512; binarized +-1 weights exact).
"""
import numpy as np

import concourse.bass as bass
import concourse.bacc as bacc
import concourse.tile as tile
from concourse.tile_rust import add_dep_helper
import concourse.mybir as mybir
from concourse.bass_utils import run_bass_kernel_spmd

N_CORES = 8
B_TOT = 16384
BPC = B_TOT // N_CORES  # 2048 batch rows per core
NB = BPC // 512  # 4 free-dim tiles of 512
D_IN, H, D_OUT = 784, 1024, 10
D_IN_PAD = 896  # pad 784 -> 7 full k-tiles of 128
KT1 = D_IN_PAD // 128
NJ = H // 128  # 8 feature tiles per hidden layer
BN_EPS = 1e-5

# stat groups per layer: [(lo, hi), (lo, hi)]
LGROUPS = {
    1: [(0, 6), (6, 8)],
    2: [(0, 6), (6, 8)],
    3: [(0, 5), (5, 8)],
}

f32 = mybir.dt.float32
f32r = mybir.dt.float32r
AF = mybir.ActivationFunctionType
ALU = mybir.AluOpType

# t_vec scratch layout (free-dim float offsets)
V_S = 0       # BN scale per feature (8)
V_T = 8       # BN shift per feature (8)
V_M = 16      # mean
V_E2 = 24
V_VU = 32     # var+eps
V_SQ = 40     # sqrt(var+eps)
V_R = 48      # rsqrt
V_TMP = 56
V_TMP2 = 64


def build(nc):
    xT = nc.dram_tensor("xT", [KT1, NB, 128, 512], f32r, kind="ExternalInput")
    w1s = nc.dram_tensor("w1s", [KT1, 128, H], f32r, kind="ExternalInput")
    w2s = nc.dram_tensor("w2s", [NJ, 128, H], f32r, kind="ExternalInput")
    w3s = nc.dram_tensor("w3s", [NJ, 128, H], f32r, kind="ExternalInput")
    w4s = nc.dram_tensor("w4s", [128, NJ, D_OUT], f32r, kind="ExternalInput")
    gbp = nc.dram_tensor("gbp", [128, 6, 8], f32, kind="ExternalInput")
    outT = nc.dram_tensor("outT", [D_OUT, BPC], f32, kind="ExternalOutput")

    rg = [list(range(N_CORES))]

    with tile.TileContext(nc) as tc:
        with (
            tc.tile_pool(name="hp", bufs=2) as hpool,
            tc.tile_pool(name="wp", bufs=2) as wpool,
            tc.tile_pool(name="w4p", bufs=1) as w4pool,
            tc.tile_pool(name="outp", bufs=2) as outpool,
            tc.tile_pool(name="msc", bufs=1) as mpool,
            tc.tile_pool(name="ps", bufs=8, space="PSUM") as pspool,
            tc.tile_pool(name="dram", bufs=1, space="DRAM") as dpool,
        ):
            t_stats = mpool.tile([128, 192], f32, name="t_stats")
            t_part = mpool.tile([128, 16], f32, name="t_part")
            t_gst = mpool.tile([128, 16], f32, name="t_gst")
            t_vec = mpool.tile([128, 72], f32, name="t_vec")
            t_gb = mpool.tile([128, 48], f32, name="t_gb")

            # --- warmup collective: absorb first-call ncfw/algorithm cost.
            with nc.named_scope("warmup_ar"):
                win = dpool.tile([128, 2], f32, name="warm_in")
                wout = dpool.tile([128, 2], f32, name="warm_out", addr_space="Shared")
                nc.gpsimd.collective_compute(
                    "AllReduce", ALU.add, replica_groups=rg,
                    ins=[win[:].opt()], outs=[wout[:].opt()],
                )

            # --- loads: x (b-major, L1's pass order) + gb + w2/w3 on sync;
            # w1 + w4 on the scalar queue so the two DMA rings run in
            # parallel during startup.  w3's WAR wait on W1s blocks only the
            # end-of-kernel output DMAs behind it.
            xT_t = hpool.tile([128, KT1, NB, 512], f32r, name="xT_t", tag="h")
            W1s = wpool.tile([128, KT1, H], f32r, name="W1s", tag="w")
            with nc.named_scope("w1x"):
                for k in range(KT1):
                    nc.scalar.dma_start(W1s[:, k, :], w1s[k])
                    nc.sync.dma_start(xT_t[:, k, 0], xT[k, 0])
            with nc.named_scope("xload"):
                for b in range(1, NB):
                    for k in range(KT1):
                        nc.sync.dma_start(xT_t[:, k, b], xT[k, b])
            nc.sync.dma_start(t_gb[:], gbp[:].rearrange("p a b -> p (a b)"))

            W4s = w4pool.tile([128, NJ, D_OUT], f32r, name="W4s")
            nc.scalar.dma_start(
                W4s[:].rearrange("p a b -> p (a b)"),
                w4s[:].rearrange("p a b -> p (a b)"),
            )

            W2s = wpool.tile([128, NJ, H], f32r, name="W2s", tag="w")
            W3s = wpool.tile([128, NJ, H], f32r, name="W3s", tag="w")
            with nc.named_scope("w23"):
                for k in range(NJ):
                    nc.sync.dma_start(W2s[:, k, :], w2s[k])
                for k in range(NJ):
                    nc.sync.dma_start(W3s[:, k, :], w3s[k])

            h1 = hpool.tile([128, NJ, NB, 512], f32r, name="h1", tag="h")
            h2 = hpool.tile([128, NJ, NB, 512], f32r, name="h2", tag="h")
            h3 = hpool.tile([128, NJ, NB, 512], f32r, name="h3", tag="h")

            def chunk_stats(j, b, acc):
                so = j * 24 + b * 6
                nc.vector.bn_stats(t_stats[:, so : so + 6], acc[:])

            def chunk_drain(out_h, j, b, acc, eng):
                if eng == "sc":
                    nc.scalar.activation(out_h[:, j, b], acc[:], AF.Copy)
                else:
                    nc.vector.tensor_copy(out_h[:, j, b], acc[:])

            def aggr_j(j):
                po = j * 2
                nc.vector.bn_aggr(
                    t_part[:, po : po + 2], t_stats[:, j * 24 : j * 24 + 24]
                )
                # E2 = var + mean^2 (AR sum then gives global E[x], E[x^2])
                nc.vector.tensor_tensor(
                    t_vec[:, V_TMP + j : V_TMP + j + 1],
                    t_part[:, po : po + 1],
                    t_part[:, po : po + 1],
                    op=ALU.mult,
                )
                nc.vector.tensor_tensor(
                    t_part[:, po + 1 : po + 2],
                    t_vec[:, V_TMP + j : V_TMP + j + 1],
                    t_part[:, po + 1 : po + 2],
                    op=ALU.add,
                )

            def collective_group(li, gi):
                j_lo, j_hi = LGROUPS[li][gi]
                n = (j_hi - j_lo) * 2
                tag = "ab"[gi]
                with nc.named_scope(f"L{li}_ar{tag}"):
                    cin = dpool.tile([128, n], f32, name=f"cin{li}{tag}")
                    cout = dpool.tile(
                        [128, n], f32, name=f"cout{li}{tag}", addr_space="Shared"
                    )
                    nc.gpsimd.dma_start(cin[:], t_part[:, j_lo * 2 : j_hi * 2])
                    nc.gpsimd.collective_compute(
                        "AllReduce", ALU.add, replica_groups=rg,
                        ins=[cin[:].opt()], outs=[cout[:].opt()],
                    )
                    # out-dma rides the sync queue: its wait on the global
                    # collective completion must not head-of-line-block the
                    # NEXT collective's cin/trigger on the gpsimd queue
                    nc.sync.dma_start(t_gst[:, j_lo * 2 : j_hi * 2], cout[:])

            def st_group(li, gi, prev_st):
                """s = g*rsqrt(v+eps), t = b - m*s for the group's feature
                tiles.  Returns (first_dve, last_dve, sqrt) for order pins."""
                j_lo, j_hi = LGROUPS[li][gi]
                tag = "ab"[gi]
                with nc.named_scope(f"L{li}_st{tag}"):
                    gview = t_gst[:, j_lo * 2 : j_hi * 2].rearrange(
                        "p (j c) -> p j c", c=2
                    )
                    mm = t_vec[:, V_M + j_lo : V_M + j_hi]
                    e2 = t_vec[:, V_E2 + j_lo : V_E2 + j_hi]
                    vu = t_vec[:, V_VU + j_lo : V_VU + j_hi]
                    sq = t_vec[:, V_SQ + j_lo : V_SQ + j_hi]
                    rr = t_vec[:, V_R + j_lo : V_R + j_hi]
                    tp2 = t_vec[:, V_TMP2 + j_lo : V_TMP2 + j_hi]
                    sv = t_vec[:, V_S + j_lo : V_S + j_hi]
                    tv = t_vec[:, V_T + j_lo : V_T + j_hi]
                    g_sl = t_gb[:, (li - 1) * 16 + j_lo : (li - 1) * 16 + j_hi]
                    b_sl = t_gb[:, (li - 1) * 16 + 8 + j_lo : (li - 1) * 16 + 8 + j_hi]
                    i0 = nc.vector.tensor_scalar(
                        mm, gview[:, :, 0], 1.0 / N_CORES, None, op0=ALU.mult
                    )
                    nc.vector.tensor_scalar(e2, gview[:, :, 1], 1.0 / N_CORES, None, op0=ALU.mult)
                    nc.vector.tensor_tensor(tp2, mm, mm, op=ALU.mult)
                    nc.vector.tensor_tensor(vu, e2, tp2, op=ALU.subtract)
                    nc.vector.tensor_scalar(vu, vu, BN_EPS, None, op0=ALU.add)
                    sq_inst = nc.scalar.activation(sq, vu, AF.Sqrt)
                    nc.vector.reciprocal(rr, sq)
                    nc.vector.tensor_tensor(sv, g_sl, rr, op=ALU.mult)
                    nc.vector.tensor_tensor(tp2, mm, sv, op=ALU.mult)
                    i_last = nc.vector.tensor_tensor(tv, b_sl, tp2, op=ALU.subtract)
                    if prev_st is not None:
                        add_dep_helper(
                            i0.ins, prev_st[1].ins, False,
                            "group-a st chain precedes group-b st on DVE",
                        )
                    return (i0, i_last, sq_inst)

            def apply_chunk(out_h, j, b, eng):
                s_ap = t_vec[:, V_S + j : V_S + j + 1]
                t_ap = t_vec[:, V_T + j : V_T + j + 1]
                if eng == "sc":
                    return nc.scalar.activation(
                        out_h[:, j, b], out_h[:, j, b].bitcast(f32),
                        AF.Relu, bias=t_ap, scale=s_ap,
                    )
                nc.vector.tensor_scalar(
                    out_h[:, j, b], out_h[:, j, b].bitcast(f32),
                    s_ap, t_ap, op0=ALU.mult, op1=ALU.add,
                )
                return nc.vector.tensor_scalar(
                    out_h[:, j, b], out_h[:, j, b].bitcast(f32),
                    0.0, None, op0=ALU.max,
                )

            def apply_js(li, out_h, js, eng, alt=False):
                last_sc = None
                with nc.named_scope(f"L{li}_ap{js[0]}{eng}"):
                    ci = 0
                    for j in js:
                        for b in range(NB):
                            e = eng if not alt else ("sc" if ci % 2 == 0 else "dve")
                            inst = apply_chunk(out_h, j, b, e)
                            if e == "sc":
                                last_sc = inst
                            ci += 1
                return last_sc

            def finish_layer(li, out_h, st_a, a_sc_last):
                """Emitted after the last feature-tile close: group-a's DVE
                apply half, then group-b AR/st/apply (tail, both engines)."""
                (a_lo, a_hi), (b_lo, b_hi) = LGROUPS[li]
                sc_hi = min(a_lo + 4, a_hi)
                if sc_hi < a_hi:
                    apply_js(li, out_h, list(range(sc_hi, a_hi)), "dve")
                collective_group(li, 1)
                st_b = st_group(li, 1, st_a)
                if a_sc_last is not None:
                    add_dep_helper(
                        st_b[2].ins, a_sc_last.ins, False,
                        "group-a applies precede group-b sqrt on ScalarE",
                    )
                apply_js(li, out_h, list(range(b_lo, b_hi)), "sc", alt=True)

            def finish_group_a(li, out_h):
                """Emitted at the group-a close: AR, st chain, and the
                ScalarE half (first 4 feature tiles) of the apply."""
                (a_lo, a_hi), _ = LGROUPS[li]
                collective_group(li, 0)
                st_a = st_group(li, 0, None)
                sc_hi = min(a_lo + 4, a_hi)
                a_sc_last = apply_js(li, out_h, list(range(a_lo, sc_hi)), "sc")
                return st_a, a_sc_last

            # ================= layer 1: batch passes, CC-bound =============
            with nc.named_scope("L1_mm"):
                for b in range(NB):
                    for j in range(NJ):
                        acc = pspool.tile(
                            [128, 512], f32, name=f"ps_l1_b{b}_j{j}", tag="ps"
                        )
                        for k in range(KT1):
                            nc.tensor.matmul(
                                acc[:],
                                W1s[:, k, j * 128 : (j + 1) * 128],
                                xT_t[:, k, b],
                                start=(k == 0),
                                stop=(k == KT1 - 1),
                            )
                        chunk_stats(j, b, acc)
                        chunk_drain(h1, j, b, acc, "sc" if j < 6 else "dve")
                for j in range(NJ):
                    aggr_j(j)
                st_a1, a_sc1 = finish_group_a(1, h1)
                finish_layer(1, h1, st_a1, a_sc1)

            def layer(li, Wcur, rhs, out_h, prev_groups):
                """Hidden layer li.  j0/j1 accumulate in k-phases matching the
                previous layer's stat groups (k-interleaved across j0/j1 so
                one applied chunk feeds two matmuls); j2..7 run straight
                k0-7 weight-stationary."""
                (_, a_hi), _ = LGROUPS[li]
                with nc.named_scope(f"L{li}_mm"):
                    accs01 = {
                        j: [
                            pspool.tile(
                                [128, 512], f32, name=f"ps_l{li}_j{j}_b{b}", tag="ps"
                            )
                            for b in range(NB)
                        ]
                        for j in (0, 1)
                    }
                    for lo, hi in prev_groups:
                        for k in range(lo, hi):
                            for j in (0, 1):
                                for b in range(NB):
                                    nc.tensor.matmul(
                                        accs01[j][b][:],
                                        Wcur[:, k, j * 128 : (j + 1) * 128],
                                        rhs[:, k, b],
                                        start=(k == 0),
                                        stop=(k == NJ - 1),
                                    )
                    for j in (0, 1):
                        for b in range(NB):
                            chunk_stats(j, b, accs01[j][b])
                        for b in range(NB):
                            chunk_drain(out_h, j, b, accs01[j][b], "sc")
                        aggr_j(j)
                    st_a = [None]
                    a_sc = [None]
                    for j in range(2, NJ):
                        accs = [
                            pspool.tile(
                                [128, 512], f32, name=f"ps_l{li}_j{j}_b{b}", tag="ps"
                            )
                            for b in range(NB)
                        ]
                        for k in range(NJ):
                            for b in range(NB):
                                nc.tensor.matmul(
                                    accs[b][:],
                                    Wcur[:, k, j * 128 : (j + 1) * 128],
                                    rhs[:, k, b],
                                    start=(k == 0),
                                    stop=(k == NJ - 1),
                                )
                        for b in range(NB):
                            chunk_stats(j, b, accs[b])
                        for b in range(NB):
                            chunk_drain(
                                out_h, j, b, accs[b], "sc" if j < a_hi else "dve"
                            )
                        aggr_j(j)
                        if j == a_hi - 1:
                            st_a[0], a_sc[0] = finish_group_a(li, out_h)
                    finish_layer(li, out_h, st_a[0], a_sc[0])

            layer(2, W2s, h1, h2, LGROUPS[1])
            layer(3, W3s, h2, h3, LGROUPS[2])

            # ---- head: 10-wide binarized linear + sigmoid, k-phase-split ----
            with nc.named_scope("L4"):
                acc4 = [
                    pspool.tile([D_OUT, 512], f32, name=f"ps_l4_b{b}", tag="ps")
                    for b in range(NB)
                ]
                for lo, hi in LGROUPS[3]:
                    for k in range(lo, hi):
                        for b in range(NB):
                            nc.tensor.matmul(
                                acc4[b][:],
                                W4s[:, k],
                                h3[:, k, b],
                                start=(k == 0),
                                stop=(k == NJ - 1),
                            )
                for b in range(NB):
                    osb = outpool.tile([D_OUT, 512], f32, name=f"osb{b}", tag="osb")
                    nc.scalar.activation(osb[:], acc4[b][:], AF.Sigmoid)
                    nc.sync.dma_start(outT[:, b * 512 : (b + 1) * 512], osb[:])

    nc.compile()
    return nc


_NC = None
_LAST_RESULTS = None


def _get_nc():
    global _NC
    if _NC is None:
        nc = bacc.Bacc(
            "TRN2", target_bir_lowering=False, debug=False, num_devices=N_CORES
        )
        build(nc)
        _NC = nc
    return _NC


def _binarize(w):
    return np.where(w >= 0, np.float32(1.0), np.float32(-1.0))


def kernel(**inputs):
    x = np.ascontiguousarray(inputs["x"], dtype=np.float32)
    w1 = np.asarray(inputs["w1"], dtype=np.float32)
    w2 = np.asarray(inputs["w2"], dtype=np.float32)
    w3 = np.asarray(inputs["w3"], dtype=np.float32)
    w4 = np.asarray(inputs["w4"], dtype=np.float32)
    gb = np.stack(
        [
            np.asarray(inputs[n], dtype=np.float32)
            for n in ("g1", "b1", "g2", "b2", "g3", "b3")
        ]
    )  # [6, 1024]

    w1sa = np.zeros((D_IN_PAD, H), np.float32)
    w1sa[:D_IN] = _binarize(w1).T
    w1sa = np.ascontiguousarray(w1sa.reshape(KT1, 128, H))
    w2sa = np.ascontiguousarray(_binarize(w2).T.reshape(NJ, 128, H))
    w3sa = np.ascontiguousarray(_binarize(w3).T.reshape(NJ, 128, H))
    w4sa = np.ascontiguousarray(
        _binarize(w4).T.reshape(NJ, 128, D_OUT).transpose(1, 0, 2)
    )  # [128, NJ, 10]
    gbp = np.ascontiguousarray(gb.reshape(6, 8, 128).transpose(2, 0, 1))  # [128,6,8]

    nc = _get_nc()
    in_maps = []
    for c in range(N_CORES):
        xs = np.zeros((D_IN_PAD, BPC), np.float32)
        xs[:D_IN] = x[c * BPC : (c + 1) * BPC].T
        xs = np.ascontiguousarray(
            xs.reshape(KT1, 128, NB, 512).transpose(0, 2, 1, 3)
        )
        in_maps.append(
            {
                "xT": xs, "w1s": w1sa, "w2s": w2sa, "w3s": w3sa,
                "w4s": w4sa, "gbp": gbp,
            }
        )

    last_err = None
    for _attempt in range(3):
        try:
            res = run_bass_kernel_spmd(nc, in_maps, core_ids=list(range(N_CORES)))
            break
        except Exception as e:  # transient NRT_EXEC_UNIT_UNRECOVERABLE etc.
            last_err = e
    else:
        raise last_err
    global _LAST_RESULTS
    _LAST_RESULTS = res
    out = np.empty((B_TOT, D_OUT), dtype=np.float32)
    for c in range(N_CORES):
        out[c * BPC : (c + 1) * BPC] = res.results[c]["outT"].T
    return out


# revision 18
# speedup vs baseline: 1.2258x; 1.2258x over previous
"""Binarized MLP forward (BinaryConnect, training-mode BatchNorm) on 8 TRN2 cores.

Strategy: data-parallel over the batch (16384 -> 8 x 2048), weights replicated.
All activations kept TRANSPOSED on device ([features, batch]) so that
 - matmuls use binarized weights as the stationary operand,
 - BatchNorm stats are free-axis reductions (bn_stats on VectorE),
 - BN apply + ReLU is a per-partition scale/bias op (ScalarE ACT / VectorE TS).

v4 schedule notes (from v1-v3 traces):
 - Weights are binarized on the HOST and shipped as +-1 f32; w1 rides the
   scalar-engine DMA queue concurrently with x on the sync queue, w2/w3 queue
   on sync BEHIND x so they cannot starve the layer-1 stream.
 - L1 runs in four batch passes (j-outer, k-inner inside a pass) so each pass
   only needs 1.6MB of x; its stats all-reduces are CC-init/skew-bound anyway.
 - Two stat groups per layer; group-a's AR triggers ~75% into the layer,
   group-b's AR rides the tail, hidden by the next layer's leading k-phase
   (first two output tiles, PSUM-bank-limited).  L3 splits (5,3) so its
   group-a AR lands earlier ahead of the small L4.
 - BN applies are produced on ScalarE and VectorE in parallel; the next
   layer's phase loop consumes k-interleaved across two output tiles so one
   applied chunk feeds two matmuls back-to-back.
 - AR-gated work is emitted after the last feature-tile close of its layer so
   engine FIFOs cannot head-of-line-block late stats behind AR waits.
Matmuls run in float32r (full PE rate at N=512; binarized +-1 weights exact).
"""
import numpy as np

import concourse.bass as bass
import concourse.bacc as bacc
import concourse.tile as tile
from concourse.tile_rust import add_dep_helper
import concourse.mybir as mybir
from concourse.bass_utils import run_bass_kernel_spmd

N_CORES = 8
B_TOT = 16384
BPC = B_TOT // N_CORES  # 2048 batch rows per core
NB = BPC // 512  # 4 free-dim tiles of 512
D_IN, H, D_OUT = 784, 1024, 10
D_IN_PAD = 896  # pad 784 -> 7 full k-tiles of 128
KT1 = D_IN_PAD // 128
NJ = H // 128  # 8 feature tiles per hidden layer
BN_EPS = 1e-5

# stat groups per layer: [(lo, hi), (lo, hi)]
LGROUPS = {
    1: [(0, 6), (6, 8)],
    2: [(0, 6), (6, 8)],
    3: [(0, 5), (5, 8)],
}

f32 = mybir.dt.float32
f32r = mybir.dt.float32r
AF = mybir.ActivationFunctionType
ALU = mybir.AluOpType

# t_vec scratch layout (free-dim float offsets)
V_S = 0       # BN scale per feature (8)
V_T = 8       # BN shift per feature (8)
V_M = 16      # mean
V_E2 = 24
V_VU = 32     # var+eps
V_SQ = 40     # sqrt(var+eps)
V_R = 48      # rsqrt
V_TMP = 56
V_TMP2 = 64


def build(nc):
    xT = nc.dram_tensor("xT", [KT1, NB, 128, 512], f32r, kind="ExternalInput")
    w1s = nc.dram_tensor("w1s", [KT1, 128, H], f32r, kind="ExternalInput")
    w2s = nc.dram_tensor("w2s", [NJ, 128, H], f32r, kind="ExternalInput")
    w3s = nc.dram_tensor("w3s", [NJ, 128, H], f32r, kind="ExternalInput")
    w4s = nc.dram_tensor("w4s", [128, NJ, D_OUT], f32r, kind="ExternalInput")
    gbp = nc.dram_tensor("gbp", [128, 6, 8], f32, kind="ExternalInput")
    outT = nc.dram_tensor("outT", [D_OUT, BPC], f32, kind="ExternalOutput")

    rg = [list(range(N_CORES))]

    with tile.TileContext(nc) as tc:
        with (
            tc.tile_pool(name="hp", bufs=2) as hpool,
            tc.tile_pool(name="wp", bufs=2) as wpool,
            tc.tile_pool(name="w4p", bufs=1) as w4pool,
            tc.tile_pool(name="outp", bufs=2) as outpool,
            tc.tile_pool(name="msc", bufs=1) as mpool,
            tc.tile_pool(name="ps", bufs=8, space="PSUM") as pspool,
            tc.tile_pool(name="dram", bufs=1, space="DRAM") as dpool,
        ):
            t_stats = mpool.tile([128, 192], f32, name="t_stats")
            t_part = mpool.tile([128, 16], f32, name="t_part")
            t_gst = mpool.tile([128, 16], f32, name="t_gst")
            t_vec = mpool.tile([128, 72], f32, name="t_vec")
            t_gb = mpool.tile([128, 48], f32, name="t_gb")

            # (no warmup collective: the mesh is gated by the slowest core's
            # CC boot either way, and a warmup only adds one more serialized
            # mesh round before L1's stats AR can complete)

            # --- loads: x (b-major, L1's pass order) + gb + w2/w3 on sync;
            # w1 + w4 on the scalar queue so the two DMA rings run in
            # parallel during startup.  w3's WAR wait on W1s blocks only the
            # end-of-kernel output DMAs behind it.
            xT_t = hpool.tile([128, KT1, NB, 512], f32r, name="xT_t", tag="h")
            W1s = wpool.tile([128, KT1, H], f32r, name="W1s", tag="w")
            with nc.named_scope("w1x"):
                for k in range(KT1):
                    nc.scalar.dma_start(W1s[:, k, :], w1s[k])
                    nc.sync.dma_start(xT_t[:, k, 0], xT[k, 0])
            with nc.named_scope("xload"):
                for b in range(1, NB):
                    for k in range(KT1):
                        nc.sync.dma_start(xT_t[:, k, b], xT[k, b])
            nc.sync.dma_start(t_gb[:], gbp[:].rearrange("p a b -> p (a b)"))

            W4s = w4pool.tile([128, NJ, D_OUT], f32r, name="W4s")
            nc.scalar.dma_start(
                W4s[:].rearrange("p a b -> p (a b)"),
                w4s[:].rearrange("p a b -> p (a b)"),
            )

            W2s = wpool.tile([128, NJ, H], f32r, name="W2s", tag="w")
            W3s = wpool.tile([128, NJ, H], f32r, name="W3s", tag="w")
            with nc.named_scope("w23"):
                for k in range(NJ):
                    nc.sync.dma_start(W2s[:, k, :], w2s[k])
                for k in range(NJ):
                    nc.sync.dma_start(W3s[:, k, :], w3s[k])

            h1 = hpool.tile([128, NJ, NB, 512], f32r, name="h1", tag="h")
            h2 = hpool.tile([128, NJ, NB, 512], f32r, name="h2", tag="h")
            h3 = hpool.tile([128, NJ, NB, 512], f32r, name="h3", tag="h")

            def chunk_stats(j, b, acc):
                so = j * 24 + b * 6
                nc.vector.bn_stats(t_stats[:, so : so + 6], acc[:])

            def chunk_drain(out_h, j, b, acc, eng):
                if eng == "sc":
                    nc.scalar.activation(out_h[:, j, b], acc[:], AF.Copy)
                else:
                    nc.vector.tensor_copy(out_h[:, j, b], acc[:])

            def aggr_j(j):
                po = j * 2
                nc.vector.bn_aggr(
                    t_part[:, po : po + 2], t_stats[:, j * 24 : j * 24 + 24]
                )
                # E2 = var + mean^2 (AR sum then gives global E[x], E[x^2])
                nc.vector.tensor_tensor(
                    t_vec[:, V_TMP + j : V_TMP + j + 1],
                    t_part[:, po : po + 1],
                    t_part[:, po : po + 1],
                    op=ALU.mult,
                )
                nc.vector.tensor_tensor(
                    t_part[:, po + 1 : po + 2],
                    t_vec[:, V_TMP + j : V_TMP + j + 1],
                    t_part[:, po + 1 : po + 2],
                    op=ALU.add,
                )

            def collective_group(li, gi):
                j_lo, j_hi = LGROUPS[li][gi]
                n = (j_hi - j_lo) * 2
                tag = "ab"[gi]
                with nc.named_scope(f"L{li}_ar{tag}"):
                    cin = dpool.tile([128, n], f32, name=f"cin{li}{tag}")
                    cout = dpool.tile(
                        [128, n], f32, name=f"cout{li}{tag}", addr_space="Shared"
                    )
                    nc.gpsimd.dma_start(cin[:], t_part[:, j_lo * 2 : j_hi * 2])
                    nc.gpsimd.collective_compute(
                        "AllReduce", ALU.add, replica_groups=rg,
                        ins=[cin[:].opt()], outs=[cout[:].opt()],
                    )
                    # out-dma rides the sync queue: its wait on the global
                    # collective completion must not head-of-line-block the
                    # NEXT collective's cin/trigger on the gpsimd queue
                    nc.sync.dma_start(t_gst[:, j_lo * 2 : j_hi * 2], cout[:])

            def st_group(li, gi, prev_st):
                """s = g*rsqrt(v+eps), t = b - m*s for the group's feature
                tiles.  Returns (first_dve, last_dve, sqrt) for order pins."""
                j_lo, j_hi = LGROUPS[li][gi]
                tag = "ab"[gi]
                with nc.named_scope(f"L{li}_st{tag}"):
                    gview = t_gst[:, j_lo * 2 : j_hi * 2].rearrange(
                        "p (j c) -> p j c", c=2
                    )
                    mm = t_vec[:, V_M + j_lo : V_M + j_hi]
                    e2 = t_vec[:, V_E2 + j_lo : V_E2 + j_hi]
                    vu = t_vec[:, V_VU + j_lo : V_VU + j_hi]
                    sq = t_vec[:, V_SQ + j_lo : V_SQ + j_hi]
                    rr = t_vec[:, V_R + j_lo : V_R + j_hi]
                    tp2 = t_vec[:, V_TMP2 + j_lo : V_TMP2 + j_hi]
                    sv = t_vec[:, V_S + j_lo : V_S + j_hi]
                    tv = t_vec[:, V_T + j_lo : V_T + j_hi]
                    g_sl = t_gb[:, (li - 1) * 16 + j_lo : (li - 1) * 16 + j_hi]
                    b_sl = t_gb[:, (li - 1) * 16 + 8 + j_lo : (li - 1) * 16 + 8 + j_hi]
                    i0 = nc.vector.tensor_scalar(
                        mm, gview[:, :, 0], 1.0 / N_CORES, None, op0=ALU.mult
                    )
                    nc.vector.tensor_scalar(e2, gview[:, :, 1], 1.0 / N_CORES, None, op0=ALU.mult)
                    nc.vector.tensor_tensor(tp2, mm, mm, op=ALU.mult)
                    nc.vector.tensor_tensor(vu, e2, tp2, op=ALU.subtract)
                    nc.vector.tensor_scalar(vu, vu, BN_EPS, None, op0=ALU.add)
                    sq_inst = nc.scalar.activation(sq, vu, AF.Sqrt)
                    nc.vector.reciprocal(rr, sq)
                    nc.vector.tensor_tensor(sv, g_sl, rr, op=ALU.mult)
                    nc.vector.tensor_tensor(tp2, mm, sv, op=ALU.mult)
                    i_last = nc.vector.tensor_tensor(tv, b_sl, tp2, op=ALU.subtract)
                    if prev_st is not None:
                        add_dep_helper(
                            i0.ins, prev_st[1].ins, False,
                            "group-a st chain precedes group-b st on DVE",
                        )
                    return (i0, i_last, sq_inst)

            def apply_chunk(out_h, j, b, eng):
                s_ap = t_vec[:, V_S + j : V_S + j + 1]
                t_ap = t_vec[:, V_T + j : V_T + j + 1]
                if eng == "sc":
                    return nc.scalar.activation(
                        out_h[:, j, b], out_h[:, j, b].bitcast(f32),
                        AF.Relu, bias=t_ap, scale=s_ap,
                    )
                nc.vector.tensor_scalar(
                    out_h[:, j, b], out_h[:, j, b].bitcast(f32),
                    s_ap, t_ap, op0=ALU.mult, op1=ALU.add,
                )
                return nc.vector.tensor_scalar(
                    out_h[:, j, b], out_h[:, j, b].bitcast(f32),
                    0.0, None, op0=ALU.max,
                )

            def apply_js(li, out_h, js, eng, alt=False, order="jb"):
                last_sc = None
                with nc.named_scope(f"L{li}_ap{js[0]}{eng}"):
                    ci = 0
                    pairs = (
                        [(j, b) for j in js for b in range(NB)]
                        if order == "jb"
                        else [(j, b) for b in range(NB) for j in js]
                    )
                    for j, b in pairs:
                        e = eng if not alt else ("sc" if ci % 2 == 0 else "dve")
                        inst = apply_chunk(out_h, j, b, e)
                        if e == "sc":
                            last_sc = inst
                        ci += 1
                return last_sc

            def finish_layer(li, out_h, st_a, a_sc_last, tail_order="jb"):
                """Emitted after the last feature-tile close: group-a's DVE
                apply half, then group-b AR/st/apply (tail, both engines)."""
                (a_lo, a_hi), (b_lo, b_hi) = LGROUPS[li]
                sc_hi = min(a_lo + 4, a_hi)
                if sc_hi < a_hi:
                    apply_js(li, out_h, list(range(sc_hi, a_hi)), "dve")
                collective_group(li, 1)
                st_b = st_group(li, 1, st_a)
                if a_sc_last is not None:
                    add_dep_helper(
                        st_b[2].ins, a_sc_last.ins, False,
                        "group-a applies precede group-b sqrt on ScalarE",
                    )
                apply_js(
                    li, out_h, list(range(b_lo, b_hi)), "sc",
                    alt=True, order=tail_order,
                )

            def finish_group_a(li, out_h):
                """Emitted at the group-a close: AR, st chain, and the
                ScalarE half (first 4 feature tiles) of the apply."""
                (a_lo, a_hi), _ = LGROUPS[li]
                collective_group(li, 0)
                st_a = st_group(li, 0, None)
                sc_hi = min(a_lo + 4, a_hi)
                a_sc_last = apply_js(li, out_h, list(range(a_lo, sc_hi)), "sc")
                return st_a, a_sc_last

            # ================= layer 1: batch passes, CC-bound =============
            with nc.named_scope("L1_mm"):
                for b in range(NB):
                    for j in range(NJ):
                        acc = pspool.tile(
                            [128, 512], f32, name=f"ps_l1_b{b}_j{j}", tag="ps"
                        )
                        for k in range(KT1):
                            nc.tensor.matmul(
                                acc[:],
                                W1s[:, k, j * 128 : (j + 1) * 128],
                                xT_t[:, k, b],
                                start=(k == 0),
                                stop=(k == KT1 - 1),
                            )
                        chunk_stats(j, b, acc)
                        chunk_drain(h1, j, b, acc, "sc" if j < 6 else "dve")
                for j in range(NJ):
                    aggr_j(j)
                st_a1, a_sc1 = finish_group_a(1, h1)
                finish_layer(1, h1, st_a1, a_sc1)

            def layer(li, Wcur, rhs, out_h, prev_groups):
                """Hidden layer li.  j0/j1 accumulate in k-phases matching the
                previous layer's stat groups (k-interleaved across j0/j1 so
                one applied chunk feeds two matmuls); j2..7 run straight
                k0-7 weight-stationary."""
                (_, a_hi), _ = LGROUPS[li]
                with nc.named_scope(f"L{li}_mm"):
                    accs01 = {
                        j: [
                            pspool.tile(
                                [128, 512], f32, name=f"ps_l{li}_j{j}_b{b}", tag="ps"
                            )
                            for b in range(NB)
                        ]
                        for j in (0, 1)
                    }
                    for lo, hi in prev_groups:
                        for k in range(lo, hi):
                            for j in (0, 1):
                                for b in range(NB):
                                    nc.tensor.matmul(
                                        accs01[j][b][:],
                                        Wcur[:, k, j * 128 : (j + 1) * 128],
                                        rhs[:, k, b],
                                        start=(k == 0),
                                        stop=(k == NJ - 1),
                                    )
                    for j in (0, 1):
                        for b in range(NB):
                            chunk_stats(j, b, accs01[j][b])
                        for b in range(NB):
                            chunk_drain(out_h, j, b, accs01[j][b], "sc")
                        aggr_j(j)
                    st_a = [None]
                    a_sc = [None]
                    for j in range(2, NJ):
                        accs = [
                            pspool.tile(
                                [128, 512], f32, name=f"ps_l{li}_j{j}_b{b}", tag="ps"
                            )
                            for b in range(NB)
                        ]
                        for k in range(NJ):
                            for b in range(NB):
                                nc.tensor.matmul(
                                    accs[b][:],
                                    Wcur[:, k, j * 128 : (j + 1) * 128],
                                    rhs[:, k, b],
                                    start=(k == 0),
                                    stop=(k == NJ - 1),
                                )
                        for b in range(NB):
                            chunk_stats(j, b, accs[b])
                        for b in range(NB):
                            chunk_drain(
                                out_h, j, b, accs[b], "sc" if j < a_hi else "dve"
                            )
                        aggr_j(j)
                        if j == a_hi - 1:
                            st_a[0], a_sc[0] = finish_group_a(li, out_h)
                    # L3's tail applies feed L4's b-major phase-B
                    finish_layer(
                        li, out_h, st_a[0], a_sc[0],
                        tail_order="bj" if li == 3 else "jb",
                    )

            layer(2, W2s, h1, h2, LGROUPS[1])
            layer(3, W3s, h2, h3, LGROUPS[2])

            # ---- head: 10-wide binarized linear + sigmoid, k-phase-split ----
            # phase-A over L3's group-a k-tiles; the tail runs b-major so each
            # batch chunk's sigmoid + output DMA fires as soon as its last
            # k-contributions land.
            with nc.named_scope("L4"):
                acc4 = [
                    pspool.tile([D_OUT, 512], f32, name=f"ps_l4_b{b}", tag="ps")
                    for b in range(NB)
                ]
                (a_lo4, a_hi4), (b_lo4, b_hi4) = LGROUPS[3]
                for k in range(a_lo4, a_hi4):
                    for b in range(NB):
                        nc.tensor.matmul(
                            acc4[b][:],
                            W4s[:, k],
                            h3[:, k, b],
                            start=(k == 0),
                            stop=(k == NJ - 1),
                        )
                for b in range(NB):
                    for k in range(b_lo4, b_hi4):
                        nc.tensor.matmul(
                            acc4[b][:],
                            W4s[:, k],
                            h3[:, k, b],
                            start=(k == 0),
                            stop=(k == NJ - 1),
                        )
                    osb = outpool.tile([D_OUT, 512], f32, name=f"osb{b}", tag="osb")
                    nc.scalar.activation(osb[:], acc4[b][:], AF.Sigmoid)
                    nc.sync.dma_start(outT[:, b * 512 : (b + 1) * 512], osb[:])

    nc.compile()
    return nc


_NC = None
_LAST_RESULTS = None


def _get_nc():
    global _NC
    if _NC is None:
        nc = bacc.Bacc(
            "TRN2", target_bir_lowering=False, debug=False, num_devices=N_CORES
        )
        build(nc)
        _NC = nc
    return _NC


def _binarize(w):
    return np.where(w >= 0, np.float32(1.0), np.float32(-1.0))


def kernel(**inputs):
    x = np.ascontiguousarray(inputs["x"], dtype=np.float32)
    w1 = np.asarray(inputs["w1"], dtype=np.float32)
    w2 = np.asarray(inputs["w2"], dtype=np.float32)
    w3 = np.asarray(inputs["w3"], dtype=np.float32)
    w4 = np.asarray(inputs["w4"], dtype=np.float32)
    gb = np.stack(
        [
            np.asarray(inputs[n], dtype=np.float32)
            for n in ("g1", "b1", "g2", "b2", "g3", "b3")
        ]
    )  # [6, 1024]

    w1sa = np.zeros((D_IN_PAD, H), np.float32)
    w1sa[:D_IN] = _binarize(w1).T
    w1sa = np.ascontiguousarray(w1sa.reshape(KT1, 128, H))
    w2sa = np.ascontiguousarray(_binarize(w2).T.reshape(NJ, 128, H))
    w3sa = np.ascontiguousarray(_binarize(w3).T.reshape(NJ, 128, H))
    w4sa = np.ascontiguousarray(
        _binarize(w4).T.reshape(NJ, 128, D_OUT).transpose(1, 0, 2)
    )  # [128, NJ, 10]
    gbp = np.ascontiguousarray(gb.reshape(6, 8, 128).transpose(2, 0, 1))  # [128,6,8]

    nc = _get_nc()
    in_maps = []
    for c in range(N_CORES):
        xs = np.zeros((D_IN_PAD, BPC), np.float32)
        xs[:D_IN] = x[c * BPC : (c + 1) * BPC].T
        xs = np.ascontiguousarray(
            xs.reshape(KT1, 128, NB, 512).transpose(0, 2, 1, 3)
        )
        in_maps.append(
            {
                "xT": xs, "w1s": w1sa, "w2s": w2sa, "w3s": w3sa,
                "w4s": w4sa, "gbp": gbp,
            }
        )

    last_err = None
    for _attempt in range(3):
        try:
            res = run_bass_kernel_spmd(nc, in_maps, core_ids=list(range(N_CORES)))
            break
        except Exception as e:  # transient NRT_EXEC_UNIT_UNRECOVERABLE etc.
            last_err = e
    else:
        raise last_err
    global _LAST_RESULTS
    _LAST_RESULTS = res
    out = np.empty((B_TOT, D_OUT), dtype=np.float32)
    for c in range(N_CORES):
        out[c * BPC : (c + 1) * BPC] = res.results[c]["outT"].T
    return out


# revision 19
# speedup vs baseline: 1.2483x; 1.0184x over previous
"""Binarized MLP forward (BinaryConnect, training-mode BatchNorm) on 8 TRN2 cores.

Strategy: data-parallel over the batch (16384 -> 8 x 2048), weights replicated.
All activations kept TRANSPOSED on device ([features, batch]) so that
 - matmuls use binarized weights as the stationary operand,
 - BatchNorm stats are free-axis reductions (bn_stats on VectorE),
 - BN apply + ReLU is a per-partition scale/bias op (ScalarE ACT / VectorE TS).

v4 schedule notes (from v1-v3 traces):
 - Weights are binarized on the HOST and shipped as +-1 f32; w1 rides the
   scalar-engine DMA queue concurrently with x on the sync queue, w2/w3 queue
   on sync BEHIND x so they cannot starve the layer-1 stream.
 - L1 runs in four batch passes (j-outer, k-inner inside a pass) so each pass
   only needs 1.6MB of x; its stats all-reduces are CC-init/skew-bound anyway.
 - Two stat groups per layer; group-a's AR triggers ~75% into the layer,
   group-b's AR rides the tail, hidden by the next layer's leading k-phase
   (first two output tiles, PSUM-bank-limited).  L3 splits (5,3) so its
   group-a AR lands earlier ahead of the small L4.
 - BN applies are produced on ScalarE and VectorE in parallel; the next
   layer's phase loop consumes k-interleaved across two output tiles so one
   applied chunk feeds two matmuls back-to-back.
 - AR-gated work is emitted after the last feature-tile close of its layer so
   engine FIFOs cannot head-of-line-block late stats behind AR waits.
Matmuls run in float32r (full PE rate at N=512; binarized +-1 weights exact).
"""
import numpy as np

import concourse.bass as bass
import concourse.bacc as bacc
import concourse.tile as tile
from concourse.tile_rust import add_dep_helper
import concourse.mybir as mybir
from concourse.bass_utils import run_bass_kernel_spmd

N_CORES = 8
B_TOT = 16384
BPC = B_TOT // N_CORES  # 2048 batch rows per core
NB = BPC // 512  # 4 free-dim tiles of 512
D_IN, H, D_OUT = 784, 1024, 10
D_IN_PAD = 896  # pad 784 -> 7 full k-tiles of 128
KT1 = D_IN_PAD // 128
NJ = H // 128  # 8 feature tiles per hidden layer
BN_EPS = 1e-5

# stat groups per layer: [(lo, hi), (lo, hi)]
LGROUPS = {
    1: [(0, 6), (6, 8)],
    2: [(0, 6), (6, 8)],
    3: [(0, 5), (5, 8)],
}

f32 = mybir.dt.float32
f32r = mybir.dt.float32r
AF = mybir.ActivationFunctionType
ALU = mybir.AluOpType

# t_vec scratch layout (free-dim float offsets)
V_S = 0       # BN scale per feature (8)
V_T = 8       # BN shift per feature (8)
V_M = 16      # mean
V_E2 = 24
V_VU = 32     # var+eps
V_SQ = 40     # sqrt(var+eps)
V_R = 48      # rsqrt
V_TMP = 56
V_TMP2 = 64


def build(nc):
    xT = nc.dram_tensor("xT", [KT1, NB, 128, 512], f32r, kind="ExternalInput")
    w1s = nc.dram_tensor("w1s", [KT1, 128, H], f32r, kind="ExternalInput")
    w2s = nc.dram_tensor("w2s", [NJ, 128, H], f32r, kind="ExternalInput")
    w3s = nc.dram_tensor("w3s", [NJ, 128, H], f32r, kind="ExternalInput")
    w4s = nc.dram_tensor("w4s", [128, NJ, D_OUT], f32r, kind="ExternalInput")
    gbp = nc.dram_tensor("gbp", [128, 6, 8], f32, kind="ExternalInput")
    outT = nc.dram_tensor("outT", [D_OUT, BPC], f32, kind="ExternalOutput")

    rg = [list(range(N_CORES))]

    with tile.TileContext(nc) as tc:
        with (
            tc.tile_pool(name="hp", bufs=2) as hpool,
            tc.tile_pool(name="wp", bufs=2) as wpool,
            tc.tile_pool(name="w4p", bufs=1) as w4pool,
            tc.tile_pool(name="outp", bufs=2) as outpool,
            tc.tile_pool(name="msc", bufs=1) as mpool,
            tc.tile_pool(name="ps", bufs=8, space="PSUM") as pspool,
            tc.tile_pool(name="dram", bufs=1, space="DRAM") as dpool,
        ):
            t_stats = mpool.tile([128, 192], f32, name="t_stats")
            t_part = mpool.tile([128, 16], f32, name="t_part")
            t_gst = mpool.tile([128, 16], f32, name="t_gst")
            t_vec = mpool.tile([128, 72], f32, name="t_vec")
            t_gb = mpool.tile([128, 48], f32, name="t_gb")

            # (no warmup collective: the mesh is gated by the slowest core's
            # CC boot either way, and a warmup only adds one more serialized
            # mesh round before L1's stats AR can complete)

            # --- loads: x (b-major, L1's pass order) + gb + w2/w3 on sync;
            # w1 + w4 on the scalar queue so the two DMA rings run in
            # parallel during startup.  w3's WAR wait on W1s blocks only the
            # end-of-kernel output DMAs behind it.
            xT_t = hpool.tile([128, KT1, NB, 512], f32r, name="xT_t", tag="h")
            W1s = wpool.tile([128, KT1, H], f32r, name="W1s", tag="w")
            with nc.named_scope("w1x"):
                for k in range(KT1):
                    nc.scalar.dma_start(W1s[:, k, :], w1s[k])
                    nc.sync.dma_start(xT_t[:, k, 0], xT[k, 0])
            with nc.named_scope("xload"):
                for b in range(1, NB):
                    for k in range(KT1):
                        nc.sync.dma_start(xT_t[:, k, b], xT[k, b])
            nc.sync.dma_start(t_gb[:], gbp[:].rearrange("p a b -> p (a b)"))

            W4s = w4pool.tile([128, NJ, D_OUT], f32r, name="W4s")
            nc.scalar.dma_start(
                W4s[:].rearrange("p a b -> p (a b)"),
                w4s[:].rearrange("p a b -> p (a b)"),
            )

            W2s = wpool.tile([128, NJ, H], f32r, name="W2s", tag="w")
            W3s = wpool.tile([128, NJ, H], f32r, name="W3s", tag="w")
            with nc.named_scope("w23"):
                for k in range(NJ):
                    nc.sync.dma_start(W2s[:, k, :], w2s[k])
                for k in range(NJ):
                    nc.sync.dma_start(W3s[:, k, :], w3s[k])

            h1 = hpool.tile([128, NJ, NB, 512], f32r, name="h1", tag="h")
            h2 = hpool.tile([128, NJ, NB, 512], f32r, name="h2", tag="h")
            h3 = hpool.tile([128, NJ, NB, 512], f32r, name="h3", tag="h")

            def chunk_stats(j, b, acc):
                so = j * 24 + b * 6
                nc.vector.bn_stats(t_stats[:, so : so + 6], acc[:])

            def chunk_drain(out_h, j, b, acc, eng):
                if eng == "sc":
                    nc.scalar.activation(out_h[:, j, b], acc[:], AF.Copy)
                else:
                    nc.vector.tensor_copy(out_h[:, j, b], acc[:])

            def aggr_j(j):
                po = j * 2
                nc.vector.bn_aggr(
                    t_part[:, po : po + 2], t_stats[:, j * 24 : j * 24 + 24]
                )
                # E2 = var + mean^2 (AR sum then gives global E[x], E[x^2])
                nc.vector.tensor_tensor(
                    t_vec[:, V_TMP + j : V_TMP + j + 1],
                    t_part[:, po : po + 1],
                    t_part[:, po : po + 1],
                    op=ALU.mult,
                )
                nc.vector.tensor_tensor(
                    t_part[:, po + 1 : po + 2],
                    t_vec[:, V_TMP + j : V_TMP + j + 1],
                    t_part[:, po + 1 : po + 2],
                    op=ALU.add,
                )

            def collective_group(li, gi):
                j_lo, j_hi = LGROUPS[li][gi]
                n = (j_hi - j_lo) * 2
                tag = "ab"[gi]
                with nc.named_scope(f"L{li}_ar{tag}"):
                    cin = dpool.tile([128, n], f32, name=f"cin{li}{tag}")
                    cout = dpool.tile(
                        [128, n], f32, name=f"cout{li}{tag}", addr_space="Shared"
                    )
                    nc.gpsimd.dma_start(cin[:], t_part[:, j_lo * 2 : j_hi * 2])
                    nc.gpsimd.collective_compute(
                        "AllReduce", ALU.add, replica_groups=rg,
                        ins=[cin[:].opt()], outs=[cout[:].opt()],
                    )
                    # out-dma rides the sync queue: its wait on the global
                    # collective completion must not head-of-line-block the
                    # NEXT collective's cin/trigger on the gpsimd queue
                    nc.sync.dma_start(t_gst[:, j_lo * 2 : j_hi * 2], cout[:])

            def st_group(li, gi, prev_st):
                """s = g*rsqrt(v+eps), t = b - m*s for the group's feature
                tiles.  Returns (first_dve, last_dve, sqrt) for order pins."""
                j_lo, j_hi = LGROUPS[li][gi]
                tag = "ab"[gi]
                with nc.named_scope(f"L{li}_st{tag}"):
                    gview = t_gst[:, j_lo * 2 : j_hi * 2].rearrange(
                        "p (j c) -> p j c", c=2
                    )
                    mm = t_vec[:, V_M + j_lo : V_M + j_hi]
                    e2 = t_vec[:, V_E2 + j_lo : V_E2 + j_hi]
                    vu = t_vec[:, V_VU + j_lo : V_VU + j_hi]
                    sq = t_vec[:, V_SQ + j_lo : V_SQ + j_hi]
                    rr = t_vec[:, V_R + j_lo : V_R + j_hi]
                    tp2 = t_vec[:, V_TMP2 + j_lo : V_TMP2 + j_hi]
                    sv = t_vec[:, V_S + j_lo : V_S + j_hi]
                    tv = t_vec[:, V_T + j_lo : V_T + j_hi]
                    g_sl = t_gb[:, (li - 1) * 16 + j_lo : (li - 1) * 16 + j_hi]
                    b_sl = t_gb[:, (li - 1) * 16 + 8 + j_lo : (li - 1) * 16 + 8 + j_hi]
                    i0 = nc.vector.tensor_scalar(
                        mm, gview[:, :, 0], 1.0 / N_CORES, None, op0=ALU.mult
                    )
                    nc.vector.tensor_scalar(e2, gview[:, :, 1], 1.0 / N_CORES, None, op0=ALU.mult)
                    nc.vector.tensor_tensor(tp2, mm, mm, op=ALU.mult)
                    nc.vector.tensor_tensor(vu, e2, tp2, op=ALU.subtract)
                    nc.vector.tensor_scalar(vu, vu, BN_EPS, None, op0=ALU.add)
                    sq_inst = nc.scalar.activation(sq, vu, AF.Sqrt)
                    nc.vector.reciprocal(rr, sq)
                    nc.vector.tensor_tensor(sv, g_sl, rr, op=ALU.mult)
                    nc.vector.tensor_tensor(tp2, mm, sv, op=ALU.mult)
                    i_last = nc.vector.tensor_tensor(tv, b_sl, tp2, op=ALU.subtract)
                    if prev_st is not None:
                        add_dep_helper(
                            i0.ins, prev_st[1].ins, False,
                            "group-a st chain precedes group-b st on DVE",
                        )
                    return (i0, i_last, sq_inst)

            def apply_chunk(out_h, j, b, eng):
                s_ap = t_vec[:, V_S + j : V_S + j + 1]
                t_ap = t_vec[:, V_T + j : V_T + j + 1]
                if eng == "sc":
                    return nc.scalar.activation(
                        out_h[:, j, b], out_h[:, j, b].bitcast(f32),
                        AF.Relu, bias=t_ap, scale=s_ap,
                    )
                nc.vector.tensor_scalar(
                    out_h[:, j, b], out_h[:, j, b].bitcast(f32),
                    s_ap, t_ap, op0=ALU.mult, op1=ALU.add,
                )
                return nc.vector.tensor_scalar(
                    out_h[:, j, b], out_h[:, j, b].bitcast(f32),
                    0.0, None, op0=ALU.max,
                )

            def apply_js(li, out_h, js, eng, alt=False, order="jb"):
                last_sc = None
                with nc.named_scope(f"L{li}_ap{js[0]}{eng}"):
                    ci = 0
                    pairs = (
                        [(j, b) for j in js for b in range(NB)]
                        if order == "jb"
                        else [(j, b) for b in range(NB) for j in js]
                    )
                    for j, b in pairs:
                        e = eng if not alt else ("sc" if ci % 2 == 0 else "dve")
                        inst = apply_chunk(out_h, j, b, e)
                        if e == "sc":
                            last_sc = inst
                        ci += 1
                return last_sc

            def finish_layer(li, out_h, st_a, a_sc_last, tail_order="jb"):
                """Emitted after the last feature-tile close: group-a's DVE
                apply half, then group-b AR/st/apply (tail, both engines)."""
                (a_lo, a_hi), (b_lo, b_hi) = LGROUPS[li]
                sc_hi = min(a_lo + 4, a_hi)
                if sc_hi < a_hi:
                    apply_js(li, out_h, list(range(sc_hi, a_hi)), "dve")
                collective_group(li, 1)
                st_b = st_group(li, 1, st_a)
                if a_sc_last is not None:
                    add_dep_helper(
                        st_b[2].ins, a_sc_last.ins, False,
                        "group-a applies precede group-b sqrt on ScalarE",
                    )
                apply_js(
                    li, out_h, list(range(b_lo, b_hi)), "sc",
                    alt=True, order=tail_order,
                )

            def finish_group_a(li, out_h):
                """Emitted at the group-a close: AR, st chain, and the
                ScalarE half (first 4 feature tiles) of the apply."""
                (a_lo, a_hi), _ = LGROUPS[li]
                collective_group(li, 0)
                st_a = st_group(li, 0, None)
                sc_hi = min(a_lo + 4, a_hi)
                a_sc_last = apply_js(li, out_h, list(range(a_lo, sc_hi)), "sc")
                return st_a, a_sc_last

            # ================= layer 1 =====================================
            # Group-a feature tiles (j0-5) complete first so their stats AR —
            # the collective every core's L2 waits on — triggers as early as
            # possible: two b-passes while x streams in, then j-major for the
            # remaining batch chunks, then the j6/j7 tail.
            def l1_chunk(j, b):
                acc = pspool.tile(
                    [128, 512], f32, name=f"ps_l1_b{b}_j{j}", tag="ps"
                )
                for k in range(KT1):
                    nc.tensor.matmul(
                        acc[:],
                        W1s[:, k, j * 128 : (j + 1) * 128],
                        xT_t[:, k, b],
                        start=(k == 0),
                        stop=(k == KT1 - 1),
                    )
                chunk_stats(j, b, acc)
                chunk_drain(h1, j, b, acc, "sc" if j < 6 else "dve")

            with nc.named_scope("L1_mm"):
                for b in (0, 1):
                    for j in range(6):
                        l1_chunk(j, b)
                for j in range(6):
                    for b in (2, 3):
                        l1_chunk(j, b)
                    aggr_j(j)
                st_a1, a_sc1 = finish_group_a(1, h1)
                for j in (6, 7):
                    for b in range(NB):
                        l1_chunk(j, b)
                    aggr_j(j)
                finish_layer(1, h1, st_a1, a_sc1)

            def layer(li, Wcur, rhs, out_h, prev_groups):
                """Hidden layer li.  j0/j1 accumulate in k-phases matching the
                previous layer's stat groups (k-interleaved across j0/j1 so
                one applied chunk feeds two matmuls); j2..7 run straight
                k0-7 weight-stationary."""
                (_, a_hi), _ = LGROUPS[li]
                with nc.named_scope(f"L{li}_mm"):
                    accs01 = {
                        j: [
                            pspool.tile(
                                [128, 512], f32, name=f"ps_l{li}_j{j}_b{b}", tag="ps"
                            )
                            for b in range(NB)
                        ]
                        for j in (0, 1)
                    }
                    for lo, hi in prev_groups:
                        for k in range(lo, hi):
                            for j in (0, 1):
                                for b in range(NB):
                                    nc.tensor.matmul(
                                        accs01[j][b][:],
                                        Wcur[:, k, j * 128 : (j + 1) * 128],
                                        rhs[:, k, b],
                                        start=(k == 0),
                                        stop=(k == NJ - 1),
                                    )
                    for j in (0, 1):
                        for b in range(NB):
                            chunk_stats(j, b, accs01[j][b])
                        for b in range(NB):
                            chunk_drain(out_h, j, b, accs01[j][b], "sc")
                        aggr_j(j)
                    st_a = [None]
                    a_sc = [None]
                    for j in range(2, NJ):
                        accs = [
                            pspool.tile(
                                [128, 512], f32, name=f"ps_l{li}_j{j}_b{b}", tag="ps"
                            )
                            for b in range(NB)
                        ]
                        for k in range(NJ):
                            for b in range(NB):
                                nc.tensor.matmul(
                                    accs[b][:],
                                    Wcur[:, k, j * 128 : (j + 1) * 128],
                                    rhs[:, k, b],
                                    start=(k == 0),
                                    stop=(k == NJ - 1),
                                )
                        for b in range(NB):
                            chunk_stats(j, b, accs[b])
                        for b in range(NB):
                            chunk_drain(
                                out_h, j, b, accs[b], "sc" if j < a_hi else "dve"
                            )
                        aggr_j(j)
                        if j == a_hi - 1:
                            st_a[0], a_sc[0] = finish_group_a(li, out_h)
                    # L3's tail applies feed L4's b-major phase-B
                    finish_layer(
                        li, out_h, st_a[0], a_sc[0],
                        tail_order="bj" if li == 3 else "jb",
                    )

            layer(2, W2s, h1, h2, LGROUPS[1])
            layer(3, W3s, h2, h3, LGROUPS[2])

            # ---- head: 10-wide binarized linear + sigmoid, k-phase-split ----
            # phase-A over L3's group-a k-tiles; the tail runs b-major so each
            # batch chunk's sigmoid + output DMA fires as soon as its last
            # k-contributions land.
            with nc.named_scope("L4"):
                acc4 = [
                    pspool.tile([D_OUT, 512], f32, name=f"ps_l4_b{b}", tag="ps")
                    for b in range(NB)
                ]
                (a_lo4, a_hi4), (b_lo4, b_hi4) = LGROUPS[3]
                for k in range(a_lo4, a_hi4):
                    for b in range(NB):
                        nc.tensor.matmul(
                            acc4[b][:],
                            W4s[:, k],
                            h3[:, k, b],
                            start=(k == 0),
                            stop=(k == NJ - 1),
                        )
                for b in range(NB):
                    for k in range(b_lo4, b_hi4):
                        nc.tensor.matmul(
                            acc4[b][:],
                            W4s[:, k],
                            h3[:, k, b],
                            start=(k == 0),
                            stop=(k == NJ - 1),
                        )
                    osb = outpool.tile([D_OUT, 512], f32, name=f"osb{b}", tag="osb")
                    nc.scalar.activation(osb[:], acc4[b][:], AF.Sigmoid)
                    nc.sync.dma_start(outT[:, b * 512 : (b + 1) * 512], osb[:])

    nc.compile()
    return nc


_NC = None
_LAST_RESULTS = None


def _get_nc():
    global _NC
    if _NC is None:
        nc = bacc.Bacc(
            "TRN2", target_bir_lowering=False, debug=False, num_devices=N_CORES
        )
        build(nc)
        _NC = nc
    return _NC


def _binarize(w):
    return np.where(w >= 0, np.float32(1.0), np.float32(-1.0))


def kernel(**inputs):
    x = np.ascontiguousarray(inputs["x"], dtype=np.float32)
    w1 = np.asarray(inputs["w1"], dtype=np.float32)
    w2 = np.asarray(inputs["w2"], dtype=np.float32)
    w3 = np.asarray(inputs["w3"], dtype=np.float32)
    w4 = np.asarray(inputs["w4"], dtype=np.float32)
    gb = np.stack(
        [
            np.asarray(inputs[n], dtype=np.float32)
            for n in ("g1", "b1", "g2", "b2", "g3", "b3")
        ]
    )  # [6, 1024]

    w1sa = np.zeros((D_IN_PAD, H), np.float32)
    w1sa[:D_IN] = _binarize(w1).T
    w1sa = np.ascontiguousarray(w1sa.reshape(KT1, 128, H))
    w2sa = np.ascontiguousarray(_binarize(w2).T.reshape(NJ, 128, H))
    w3sa = np.ascontiguousarray(_binarize(w3).T.reshape(NJ, 128, H))
    w4sa = np.ascontiguousarray(
        _binarize(w4).T.reshape(NJ, 128, D_OUT).transpose(1, 0, 2)
    )  # [128, NJ, 10]
    gbp = np.ascontiguousarray(gb.reshape(6, 8, 128).transpose(2, 0, 1))  # [128,6,8]

    nc = _get_nc()
    in_maps = []
    for c in range(N_CORES):
        xs = np.zeros((D_IN_PAD, BPC), np.float32)
        xs[:D_IN] = x[c * BPC : (c + 1) * BPC].T
        xs = np.ascontiguousarray(
            xs.reshape(KT1, 128, NB, 512).transpose(0, 2, 1, 3)
        )
        in_maps.append(
            {
                "xT": xs, "w1s": w1sa, "w2s": w2sa, "w3s": w3sa,
                "w4s": w4sa, "gbp": gbp,
            }
        )

    last_err = None
    for _attempt in range(3):
        try:
            res = run_bass_kernel_spmd(nc, in_maps, core_ids=list(range(N_CORES)))
            break
        except Exception as e:  # transient NRT_EXEC_UNIT_UNRECOVERABLE etc.
            last_err = e
    else:
        raise last_err
    global _LAST_RESULTS
    _LAST_RESULTS = res
    out = np.empty((B_TOT, D_OUT), dtype=np.float32)
    for c in range(N_CORES):
        out[c * BPC : (c + 1) * BPC] = res.results[c]["outT"].T
    return out
